# revision 42
# baseline (speedup 1.0000x reference)
"""AttentiveStatPooling Trainium2 kernel (8-core SPMD, data-parallel over batch).

Contract: kernel(**inputs) takes the FULL unsharded inputs (as produced by
reference.setup_inputs()) and returns the FULL [B, 2C] output.

Math (per sample, identical to the jax reference):
  mean/std over T of x;  h = relu(Wx@x + (Wm@mean + Ws@std + b1));
  g = tanh(BN1(h));  l = inv2 * relu(W2@g + b2)  (the BN2 shift cancels in
  the softmax and is dropped);  w = softmax(l, axis=T);
  out = [sum(x*w), sqrt(clip(sum(x^2*w) - mu^2, 1e-4))].

Implementation notes (v5 — LP-balanced engine assignment, walrus-legal ops):
  - batch 32 split 4 samples/core across 8 NeuronCores (pure DP).
  - x shipped in bf16, pre-transposed host-side to partition-major layout so
    every x DMA is one contiguous run per partition (128 descriptors; the SP
    sequencer issues descriptors serially at ~9 ns each).
  - inv2 is folded into W2 host-side and the exp bias is eliminated: with
    E' = exp(inv2*W2g) and thr[c] = exp(-inv2[c]*b2[c]),
    max(E', thr) = exp(-inv2*b2) * exp(inv2*relu(W2g + b2)); the
    per-channel factor cancels in mu = S1/S0 and sg.  So the exp needs no
    bias/scale and eb folds the relu via a per-partition max threshold.
  - per-chunk phase C (engine knobs tuned to the TimelineSim cost model):
      eb = max(E',thr)  DVE ts 4x, accum -> S0            (321 ns)
      pt = eb*x         DVE tt 2x                          (581 ns)
      S1 = sum(pt)      DVE ts 4x accum                    (321 ns)
      qt = pt*x         Pool tt (2079 ns) ~10/12 chunks, DVE tt else
      S2 = sum(qt)      DVE ts accum (321) or ACT Identity+accum (1205),
                        deferred two chunks so nothing waits on the Pool
  - phase A moments: sum(x) DVE ts 4x accum; x^2+sum(x^2) mostly ACT
    (Square+accum), some Pool tt + DVE accum, per knob strings.
    (Pool supports only plain tensor_tensor / tensor_scalar without accum —
    anything fancier fails the walrus ISA check on the Pool engine.)
  - front samples (carrying phase A of s+2) balance at ~2.18 us/chunk,
    tail samples at ~1.69 us/chunk on DVE/ACT/Pool.
  - sqrt via Newton/rsqrt on the vector engine (ACT Sqrt lives in a
    different activation-table set; a switch costs 1283 ns).
  - mu and sg are packed into one [128, 24] tile and leave through a single
    PE transpose + copy + one 12 KiB DMA per sample.
  - software pipeline: phase A of sample s+2 and phase B of sample s+1
    interleave into phase C of sample s; mm1(s+2) reuses the single ph1
    PSUM bank after relu(s+1) reads it; the previous sample's output stage
    defers into the next sample's chunks.
"""

import numpy as np
import ml_dtypes

B, C, T, A = 32, 1536, 1000, 128
N_CORES = 8
SPC = B // N_CORES        # samples per core
NCH = C // 128            # 12 channel chunks of 128
BN_EPS = 1e-5
CLAMP = 1e-4
HALVES = ((0, 512), (512, 1000))             # psum-bank-aligned split of T
PAIRH = ((0, 512), (512, 1000))

_CACHE = {}

# Engine-balance knobs (index = c % 12).
#   QT: qt engine: 'P' (Pool tensor_tensor) or 'D' (DVE tt)
#   S2: S2 accumulate engine: 'D' (DVE ts 4x) or 'A' (ACT Identity+accum)
#   X2: phase-A x^2+sum(x^2): 'A' (ACT Square+accum), 'P' (Pool tt + DVE
#       accum), 'D' (DVE tt + ts accum)
FRONT_QT = "PPPPPPPPPPPP"   # all Pool
FRONT_S2 = "DDDDDDDDDDDD"
FRONT_X2 = "AAAAADAADAAD"   # A=9 D=3
TAIL_QT = "PPPPPDPPPPPD"    # P=10 D=2
TAIL_S2 = "DADADADDADDA"    # D=7 A=5 (P-chunks only; D-chunks inline)
LAST_S1 = "DDDDDDDDDAAA"    # last sample: S1 accum moves to the otherwise-
                            # idle ACT for the trailing chunks
PRO_X2 = "APAAPAAPAAPA"     # prologue samples 0/1: A=8 P=4 (no DVE x^2)


def _build_module(loop_reps=1):
    import concourse.tile as tile
    from concourse import bacc, mybir
    from contextlib import ExitStack

    f32, bf16 = mybir.dt.float32, mybir.dt.bfloat16
    Alu = mybir.AluOpType
    Act = mybir.ActivationFunctionType

    nc = bacc.Bacc("TRN2", target_bir_lowering=False, debug=False,
                   num_devices=N_CORES)

    xbf = nc.dram_tensor("xbf", [SPC, 128, NCH * T], bf16, kind="ExternalInput").ap()
    w1xT = nc.dram_tensor("w1xT", [128, NCH * A], bf16, kind="ExternalInput").ap()
    wmsT = nc.dram_tensor("wmsT", [128, 2 * NCH * A], f32, kind="ExternalInput").ap()
    w2T = nc.dram_tensor("w2T", [A, C], bf16, kind="ExternalInput").ap()
    b1d = nc.dram_tensor("b1d", [A, 1], f32, kind="ExternalInput").ap()
    inv1d = nc.dram_tensor("inv1d", [A, 1], f32, kind="ExternalInput").ap()
    add1d = nc.dram_tensor("add1d", [A, 1], f32, kind="ExternalInput").ap()
    thrd = nc.dram_tensor("thrd", [128, NCH], f32, kind="ExternalInput").ap()
    identd = nc.dram_tensor("identd", [128, 128], f32, kind="ExternalInput").ap()
    out = nc.dram_tensor("out", [SPC, 2 * C], f32, kind="ExternalOutput").ap()

    with tile.TileContext(nc) as tc:
        with ExitStack() as ctx:
            cpool = ctx.enter_context(tc.tile_pool(name="const", bufs=1))
            xpool = ctx.enter_context(tc.tile_pool(name="x", bufs=14))
            epool = ctx.enter_context(tc.tile_pool(name="e", bufs=3))
            ebpool = ctx.enter_context(tc.tile_pool(name="eb", bufs=3))
            ppool = ctx.enter_context(tc.tile_pool(name="p", bufs=3))
            qpool = ctx.enter_context(tc.tile_pool(name="q", bufs=6))
            jpool = ctx.enter_context(tc.tile_pool(name="junk", bufs=6))
            rpool = ctx.enter_context(tc.tile_pool(name="r", bufs=2))
            gpool = ctx.enter_context(tc.tile_pool(name="g", bufs=2))
            spool = ctx.enter_context(tc.tile_pool(name="stats", bufs=4))
            smpool = ctx.enter_context(tc.tile_pool(name="small", bufs=8))
            opool = ctx.enter_context(tc.tile_pool(name="ostage", bufs=3))
            ph1p = ctx.enter_context(tc.tile_pool(name="ph1", bufs=1, space="PSUM"))
            p2p = ctx.enter_context(tc.tile_pool(name="p2", bufs=2, space="PSUM"))
            pmvp = ctx.enter_context(tc.tile_pool(name="pmv", bufs=1, space="PSUM"))
            ptrp = ctx.enter_context(tc.tile_pool(name="ptr", bufs=1, space="PSUM"))

            st = {}   # per-sample state

            def dma_x(s, groups=range(4), split_first=False, eng=None):
                if s not in st:
                    st[s] = {"xg": [], "x": []}
                issuer = eng if eng is not None else nc.sync
                for g in groups:
                    xt = xpool.tile([128, 3 * T], bf16, name="x", tag="x")
                    src_ap = xbf[s, :, g * 3 * T:(g + 1) * 3 * T]
                    if split_first and g == 0:
                        # land chunk 0 first so phase A can start sooner
                        issuer.dma_start(xt[:, 0:T], src_ap[:, 0:T])
                        issuer.dma_start(xt[:, T:3 * T], src_ap[:, T:3 * T])
                    else:
                        issuer.dma_start(xt[:], src_ap)
                    st[s]["xg"].append(xt)
                    for i in range(3):
                        st[s]["x"].append(xt[:, i * T:(i + 1) * T])

            def phaseA_init(s):
                d = st[s]
                if "Mx" not in d:
                    d["Mx"] = spool.tile([128, NCH], f32, name="Mx", tag="Mx")
                    d["Mx2"] = spool.tile([128, NCH], f32, name="Mx2", tag="Mx2")

            def phaseA_sumx(s, c):
                """sum(x) on DVE (ts 4x accum)."""
                d = st[s]
                phaseA_init(s)
                xt = d["x"][c]
                j0 = jpool.tile([128, T], bf16, name="junk", tag="junk")
                nc.vector.tensor_scalar(j0[:], xt, 0.0, 0.0, Alu.add, Alu.add,
                                        accum_out=d["Mx"][:, c:c + 1])

            def phaseA_x2(s, c, x2):
                """x^2+sum(x^2): ACT Square+accum / Pool tt + DVE accum /
                DVE tt + ts accum."""
                d = st[s]
                phaseA_init(s)
                xt = d["x"][c]
                if x2 == "A":
                    j1 = jpool.tile([128, T], bf16, name="junk", tag="junk")
                    nc.scalar.activation(j1[:], xt, Act.Square,
                                         accum_out=d["Mx2"][:, c:c + 1])
                else:
                    x2t = jpool.tile([128, T], bf16, name="junk", tag="junk")
                    if x2 == "P":
                        nc.gpsimd.tensor_tensor(x2t[:], xt, xt, Alu.mult)
                    else:
                        nc.vector.tensor_tensor(x2t[:], xt, xt, Alu.mult)
                    j1 = jpool.tile([128, T], bf16, name="junk", tag="junk")
                    nc.vector.tensor_scalar(j1[:], x2t[:], 0.0, 0.0, Alu.add,
                                            Alu.add,
                                            accum_out=d["Mx2"][:, c:c + 1])

            def phaseA_moments(s, c, x2="A"):
                phaseA_sumx(s, c)
                phaseA_x2(s, c, x2)

            def phaseA_mm1(s, c):
                d = st[s]
                if c == 0:
                    d["ph1"] = ph1p.tile([A, T], f32, name="ph1", tag="ph1")
                xt = d["x"][c]
                for lo, hi in HALVES:
                    nc.tensor.matmul(d["ph1"][:, lo:hi], w1xT_t[c],
                                     xt[:, lo:hi], start=(c == 0),
                                     stop=(c == NCH - 1), skip_group_check=True)

            def newton_rsqrt(v_ap, out_ap, n, iters):
                """out = 1/sqrt(v) elementwise on a [128, n] fp32 AP.
                Seed r0 = 2/(1+v), then Newton r' = r*(1.5 - 0.5*v*r^2)."""
                t0 = smpool.tile([128, n], f32, name="nw0", tag="nw0")
                t1 = smpool.tile([128, n], f32, name="nw1", tag="nw1")
                r = smpool.tile([128, n], f32, name="nwr", tag="nwr")
                nc.vector.tensor_scalar(t0[:], v_ap, 0.5, 0.5, Alu.mult, Alu.add)
                nc.vector.reciprocal(r[:], t0[:])
                for it in range(iters):
                    dst = out_ap if it == iters - 1 else r[:]
                    nc.vector.tensor_tensor(t0[:], r[:], r[:], Alu.mult)
                    nc.vector.scalar_tensor_tensor(t1[:], t0[:], -0.5, v_ap,
                                                   Alu.mult, Alu.mult)
                    nc.vector.scalar_tensor_tensor(dst, t1[:], 1.5, r[:],
                                                   Alu.add, Alu.mult)

            def phaseB_stats1(s):
                """mean + mean-half of the bias matvec."""
                d = st[s]
                meanc = smpool.tile([128, NCH], f32, name="meanc", tag="meanc")
                nc.vector.tensor_scalar(meanc[:], d["Mx"][:], 1.0 / T, None, Alu.mult)
                d["meanc"] = meanc
                pmv = pmvp.tile([A, 1], f32, name="pmv", tag="pmv")
                d["pmv"] = pmv
                for k in range(NCH):
                    nc.tensor.matmul(pmv[:], wms_t[k], meanc[:, k:k + 1],
                                     start=(k == 0), stop=False,
                                     skip_group_check=True)

            def phaseB_stats2(s):
                """variance + Newton seed.  unbiased var =
                (Mx2 - T*mean^2)/(T-1);  T*mean^2 = mean*Mx."""
                d = st[s]
                meanc = d["meanc"]
                tm2 = smpool.tile([128, NCH], f32, name="tm2", tag="tm2")
                nc.vector.tensor_tensor(tm2[:], meanc[:], d["Mx"][:], Alu.mult)
                vdiff = smpool.tile([128, NCH], f32, name="vdiff", tag="vdiff")
                nc.vector.scalar_tensor_tensor(vdiff[:], tm2[:], -1.0, d["Mx2"][:],
                                               Alu.mult, Alu.add)
                v = smpool.tile([128, NCH], f32, name="v", tag="v")
                nc.vector.tensor_scalar(v[:], vdiff[:], 1.0 / (T - 1.0), CLAMP,
                                        Alu.mult, Alu.max)
                d["v"] = v
                t0 = smpool.tile([128, NCH], f32, name="nw0", tag="nw0")
                nc.vector.tensor_scalar(t0[:], v[:], 0.5, 0.5, Alu.mult, Alu.add)
                r = smpool.tile([128, NCH], f32, name="nwr", tag="nwr")
                nc.vector.reciprocal(r[:], t0[:])
                d["r"] = r

            def phaseB_stats3(s):
                """one Newton iteration + std."""
                d = st[s]
                v, r = d["v"], d["r"]
                t0 = smpool.tile([128, NCH], f32, name="nw0", tag="nw0")
                t1 = smpool.tile([128, NCH], f32, name="nw1", tag="nw1")
                rs = smpool.tile([128, NCH], f32, name="rs", tag="rs")
                nc.vector.tensor_tensor(t0[:], r[:], r[:], Alu.mult)
                nc.vector.scalar_tensor_tensor(t1[:], t0[:], -0.5, v[:],
                                               Alu.mult, Alu.mult)
                nc.vector.scalar_tensor_tensor(rs[:], t1[:], 1.5, r[:],
                                               Alu.add, Alu.mult)
                std_t = smpool.tile([128, NCH], f32, name="std_t", tag="std_t")
                d["std_t"] = std_t
                nc.vector.tensor_tensor(std_t[:], v[:], rs[:], Alu.mult)

            def phaseB_stats(s):
                phaseB_stats1(s)
                phaseB_stats2(s)
                phaseB_stats3(s)

            def phaseB_matvec2(s):
                """std-half of the matvec + btot."""
                d = st[s]
                pmv, std_t = d["pmv"], d["std_t"]
                for k in range(NCH):
                    nc.tensor.matmul(pmv[:], wms_t[NCH + k], std_t[:, k:k + 1],
                                     start=False, stop=(k == NCH - 1),
                                     skip_group_check=True)
                btot = smpool.tile([A, 1], f32, name="btot", tag="btot")
                nc.vector.tensor_tensor(btot[:], pmv[:], b1_t[:], Alu.add)
                d["btot"] = btot

            def phaseB_relu(s):
                d = st[s]
                rt = rpool.tile([A, T], bf16, name="r", tag="r")
                nc.scalar.activation(rt[:], d["ph1"][:], Act.Relu, bias=d["btot"][:])
                d["rt"] = rt

            def phaseB_tanh(s):
                d = st[s]
                gt = gpool.tile([A, T], bf16, name="g", tag="g")
                nc.scalar.activation(gt[:], d["rt"][:], Act.Tanh, bias=add1_t[:],
                                     scale=inv1_t[:])
                d["g"] = gt

            def phaseB(s):
                phaseB_stats(s)
                phaseB_matvec2(s)
                phaseB_relu(s)
                phaseB_tanh(s)

            def phaseC_mm2exp(s, c):
                """PE matmul2 + ACT exp for chunk c (double-buffered p2, so
                mm2(c+1) overlaps exp(c)).  inv2 folded into w2T; no bias."""
                d = st[s]
                if c == 0:
                    d["S0"] = spool.tile([128, NCH], f32, name="S0", tag="S0")
                    d["S1"] = spool.tile([128, NCH], f32, name="S1", tag="S1")
                    d["S2"] = spool.tile([128, NCH], f32, name="S2", tag="S2")
                    d["E"] = [None] * NCH
                p2 = p2p.tile([128, T], f32, name="p2", tag="p2")
                wsl = w2T_t[:, c * 128:(c + 1) * 128]
                for lo, hi in HALVES:
                    nc.tensor.matmul(p2[:, lo:hi], wsl, d["g"][:, lo:hi],
                                     start=True, stop=True)
                E = epool.tile([128, T], bf16, name="E", tag="E")
                nc.scalar.activation(E[:], p2[:], Act.Exp)
                d["E"][c] = E

            def phaseC_dve(s, c, qt_pool=True, s1_eng="D"):
                """eb/S0, pt, S1 on DVE; qt on Pool tt (or DVE tt).  The S2
                accumulate is deferred (phaseC_s2)."""
                d = st[s]
                Ec = d["E"][c]
                d["E"][c] = None
                eb = ebpool.tile([128, T], bf16, name="eb", tag="eb")
                nc.vector.tensor_scalar(eb[:], Ec, thr_t[:, c:c + 1], 0.0,
                                        Alu.max, Alu.add,
                                        accum_out=d["S0"][:, c:c + 1])
                xt = d["x"][c]
                pt = ppool.tile([128, T], bf16, name="p", tag="p")
                nc.vector.tensor_tensor(pt[:], eb[:], xt, Alu.mult)
                j1 = jpool.tile([128, T], bf16, name="junk", tag="junk")
                if s1_eng == "A":
                    nc.scalar.activation(j1[:], pt[:], Act.Identity,
                                         accum_out=d["S1"][:, c:c + 1])
                else:
                    nc.vector.tensor_scalar(j1[:], pt[:], 0.0, 0.0, Alu.add,
                                            Alu.add,
                                            accum_out=d["S1"][:, c:c + 1])
                qt = qpool.tile([128, T], bf16, name="q", tag="q")
                if qt_pool:
                    nc.gpsimd.tensor_tensor(qt[:], pt[:], xt, Alu.mult)
                    d["qt_%d" % c] = qt
                else:
                    # DVE qt: the S2 accumulate can follow immediately — the
                    # in-order DVE never stalls on its own output.
                    nc.vector.tensor_tensor(qt[:], pt[:], xt, Alu.mult)
                    j2 = jpool.tile([128, T], bf16, name="junk", tag="junk")
                    nc.vector.tensor_scalar(j2[:], qt[:], 0.0, 0.0, Alu.add,
                                            Alu.add,
                                            accum_out=d["S2"][:, c:c + 1])

            def phaseC_s2(s, c, eng="D"):
                """S2 accumulate for chunk c (deferred; reads qt)."""
                d = st[s]
                qt = d.pop("qt_%d" % c)
                j2 = jpool.tile([128, T], bf16, name="junk", tag="junk")
                if eng == "A":
                    nc.scalar.activation(j2[:], qt[:], Act.Identity,
                                         accum_out=d["S2"][:, c:c + 1])
                else:
                    nc.vector.tensor_scalar(j2[:], qt[:], 0.0, 0.0, Alu.add,
                                            Alu.add,
                                            accum_out=d["S2"][:, c:c + 1])

            def store_sample(s):
                """One PE transpose of the packed [128, 24] mu|sg tile, one
                copy, one 12 KiB DMA."""
                d = st[s]
                ptr = ptrp.tile([2 * NCH, 128], f32, name="ptr", tag="ptr")
                nc.tensor.transpose(ptr[:], d["musg"][:], ident_t[:])
                ost = opool.tile([2 * NCH, 128], f32, name="ost", tag="ost")
                nc.scalar.copy(ost[:], ptr[:])
                dst = out[s, :].rearrange("(ci p) -> ci p", p=128)
                nc.sync.dma_start(dst, ost[:])

            def store_last_half(s, half):
                """Epilogue variant: ship mu while the sg chain still runs.
                Rows [0:12] / [12:24] of the shared ptr tile hold the two
                halves (disjoint partitions, no WAR)."""
                d = st[s]
                if "lptr" not in d:
                    d["lptr"] = ptrp.tile([2 * NCH, 128], f32, name="ptr", tag="ptr")
                ptr = d["lptr"]
                lo, hi = half * NCH, (half + 1) * NCH
                # transpose output base partition must be 0/32/64: reuse rows
                # [0:12] for both halves (the mu copy drains before sg's
                # transpose, so the WAR just serializes briefly).
                nc.tensor.transpose(ptr[0:NCH, :], d["musg"][:, lo:hi], ident_t[:])
                ost = opool.tile([NCH, 128], f32, name="osth", tag="osth")
                nc.scalar.copy(ost[:], ptr[0:NCH, :])
                dst = out[s, half * C:(half + 1) * C]
                dst = dst.rearrange("(ci p) -> ci p", p=128)
                nc.sync.dma_start(dst, ost[:])

            def sample_out_mu(s):
                """mu (needs only S0/S1) into musg[:, :12]."""
                d = st[s]
                rc = smpool.tile([128, NCH], f32, name="rc", tag="rc")
                nc.vector.reciprocal(rc[:], d["S0"][:])
                d["rc"] = rc
                musg = opool.tile([128, 2 * NCH], f32, name="musg", tag="musg")
                d["musg"] = musg
                nc.vector.tensor_tensor(musg[:, 0:NCH], d["S1"][:], rc[:], Alu.mult)

            def sample_out_sg1(s):
                """weighted second moment -> variance + Newton seed."""
                d = st[s]
                rc, musg = d["rc"], d["musg"]
                mu = musg[:, 0:NCH]
                ex2 = smpool.tile([128, NCH], f32, name="ex2", tag="ex2")
                nc.vector.tensor_tensor(ex2[:], d["S2"][:], rc[:], Alu.mult)
                mu2 = smpool.tile([128, NCH], f32, name="mu2", tag="mu2")
                nc.vector.tensor_tensor(mu2[:], mu, mu, Alu.mult)
                sg2 = smpool.tile([128, NCH], f32, name="sg2", tag="sg2")
                nc.vector.scalar_tensor_tensor(sg2[:], mu2[:], -1.0, ex2[:],
                                               Alu.mult, Alu.add)
                v2 = smpool.tile([128, NCH], f32, name="v2", tag="v2")
                nc.vector.tensor_scalar(v2[:], sg2[:], 1.0, CLAMP, Alu.mult, Alu.max)
                d["v2"] = v2
                t0 = smpool.tile([128, NCH], f32, name="nw0", tag="nw0")
                nc.vector.tensor_scalar(t0[:], v2[:], 0.5, 0.5, Alu.mult, Alu.add)
                r2 = smpool.tile([128, NCH], f32, name="nwr", tag="nwr")
                nc.vector.reciprocal(r2[:], t0[:])
                d["r2"] = r2

            def sample_out_sg2(s, store=True):
                """one Newton iteration + sg + store."""
                d = st[s]
                musg, v2, r2 = d["musg"], d["v2"], d["r2"]
                t0 = smpool.tile([128, NCH], f32, name="nw0", tag="nw0")
                t1 = smpool.tile([128, NCH], f32, name="nw1", tag="nw1")
                rsg = smpool.tile([128, NCH], f32, name="rsg", tag="rsg")
                nc.vector.tensor_tensor(t0[:], r2[:], r2[:], Alu.mult)
                nc.vector.scalar_tensor_tensor(t1[:], t0[:], -0.5, v2[:],
                                               Alu.mult, Alu.mult)
                nc.vector.scalar_tensor_tensor(rsg[:], t1[:], 1.5, r2[:],
                                               Alu.add, Alu.mult)
                nc.vector.tensor_tensor(musg[:, NCH:2 * NCH], v2[:], rsg[:], Alu.mult)
                if store:
                    store_sample(s)

            def sample_out_sg(s, iters=1):
                sample_out_sg1(s)
                sample_out_sg2(s)

            # ---------------- constant loads (interleaved with x below) ----
            def load_w1xT():
                t = cpool.tile([128, NCH * A], bf16, name="w1xall", tag="w1xall")
                nc.sync.dma_start(t[:], w1xT[:])
                return [t[:, c * A:(c + 1) * A] for c in range(NCH)]

            def load_params():
                global b1_t, inv1_t, add1_t, thr_t, w2T_t, wms_t, ident_t
                b1_t = cpool.tile([A, 1], f32, name="b1", tag="b1")
                nc.sync.dma_start(b1_t[:], b1d[:])
                inv1_t = cpool.tile([A, 1], f32, name="inv1", tag="inv1")
                nc.sync.dma_start(inv1_t[:], inv1d[:])
                add1_t = cpool.tile([A, 1], f32, name="add1", tag="add1")
                nc.sync.dma_start(add1_t[:], add1d[:])
                thr_t = cpool.tile([128, NCH], f32, name="thr", tag="thr")
                nc.sync.dma_start(thr_t[:], thrd[:])
                w2T_t = cpool.tile([A, C], bf16, name="w2T", tag="w2T")
                nc.sync.dma_start(w2T_t[:], w2T[:])
                ident_t = cpool.tile([128, 128], f32, name="ident", tag="ident")
                nc.sync.dma_start(ident_t[:], identd[:])
                wt = cpool.tile([128, 2 * NCH * A], f32, name="wmsall", tag="wmsall")
                nc.sync.dma_start(wt[:], wmsT[:])
                wms_t = [wt[:, k * A:(k + 1) * A] for k in range(2 * NCH)]

            def body():
                global w1xT_t
                # prologue: phase A of samples 0/1, weights interleaved,
                # sample 2's DMA prefetched.  x^2 rotates over ACT/Pool/DVE.
                dma_x(0, groups=[0], split_first=True)
                w1xT_t = load_w1xT()
                dma_x(0, groups=[1, 2, 3])
                for c in range(NCH):
                    phaseA_moments(0, c, x2=PRO_X2[c])
                    phaseA_mm1(0, c)
                load_params()
                dma_x(1)
                for c in range(NCH):
                    phaseA_moments(1, c, x2=PRO_X2[(c + 5) % NCH])
                dma_x(2)
                phaseB(0)
                # steady state: C(s) carries A(s+2) moments, B(s+1) spread
                # over bc slots, and A(s+2)'s matmul1 in the c>=8 shadow of
                # relu(s+1) freeing the ph1 slot.  mm2+exp run two chunks
                # ahead of the DVE consumer.
                mm2exp_seq = [(s, c) for s in range(SPC) for c in range(NCH)]
                mm2exp_pos = 0

                def emit_mm2exp_upto(i):
                    nonlocal mm2exp_pos
                    while mm2exp_pos <= i and mm2exp_pos < len(mm2exp_seq):
                        phaseC_mm2exp(*mm2exp_seq[mm2exp_pos])
                        mm2exp_pos += 1

                def a_tasks(s, c):
                    """Phase-A (sample, chunk) work carried by chunk (s, c)."""
                    out_t = []
                    if s == 2 and c < 4:
                        out_t.append((3, 8 + c))
                    if s + 2 < SPC:
                        if s == 0:
                            # x(2)'s DMA was only issued in the prologue:
                            # shift A(2) two chunks later so the in-order
                            # DVE stream doesn't park on it.
                            if c >= 2:
                                out_t.append((2, c - 2))
                        elif s == 1:
                            # A(3) spills 4 chunks into C(2) to even the
                            # front/tail engine loads.
                            if c < 8:
                                out_t.append((3, c))
                            if c < 2:
                                out_t.append((2, 10 + c))
                        else:
                            out_t.append((s + 2, c))
                    return out_t

                emit_mm2exp_upto(1)
                for c in range(NCH):
                    phaseA_mm1(1, c)
                prev_qt_map = None
                for s in range(SPC):
                    has_a = s + 2 < SPC          # phase-A work interleaved?
                    qt_map = FRONT_QT if has_a else TAIL_QT
                    s2_map = FRONT_S2 if has_a else TAIL_S2
                    s1_map = "D" * NCH
                    if s == SPC - 1:
                        qt_map = qt_map[:NCH - 2] + "DD"
                        s2_map = s2_map[:NCH - 2] + "DD"
                        s1_map = LAST_S1
                    for c in range(NCH):
                        tasks = a_tasks(s, c)
                        emit_mm2exp_upto(s * NCH + c + 2)
                        phaseC_dve(s, c, qt_pool=(qt_map[c] == "P"),
                                   s1_eng=s1_map[c])
                        if c > 2 and qt_map[c - 3] == "P":
                            phaseC_s2(s, c - 3, eng=s2_map[c - 3])
                        # deferred flush of the previous sample: its last S2s
                        # and the output stage run here, hidden under this
                        # sample's chunk stream.
                        if c == 0 and s - 1 in st:
                            if prev_qt_map[NCH - 3] == "P":
                                phaseC_s2(s - 1, NCH - 3, eng="D")
                            sample_out_mu(s - 1)
                        if c == 1 and s - 1 in st:
                            if prev_qt_map[NCH - 2] == "P":
                                phaseC_s2(s - 1, NCH - 2, eng="D")
                            if prev_qt_map[NCH - 1] == "P":
                                phaseC_s2(s - 1, NCH - 1, eng="D")
                        if c == 2 and s - 1 in st:
                            sample_out_sg1(s - 1)
                        if c == 3 and s - 1 in st:
                            sample_out_sg2(s - 1)
                            del st[s - 1]
                        if s + 3 < SPC and c == 0:
                            dma_x(s + 3)
                        for ss, cc in tasks:
                            phaseA_sumx(ss, cc)
                            phaseA_x2(ss, cc, FRONT_X2[cc])
                        bc = (4, 5, 6, 7, 8, 9) if s == 2 else (3, 4, 5, 6, 7, 8)
                        if s + 1 < SPC:
                            if c == bc[0]:
                                phaseB_stats1(s + 1)
                            elif c == bc[1]:
                                phaseB_stats2(s + 1)
                            elif c == bc[2]:
                                phaseB_stats3(s + 1)
                            elif c == bc[3]:
                                phaseB_matvec2(s + 1)
                            elif c == bc[4]:
                                phaseB_relu(s + 1)
                            elif c == bc[5]:
                                phaseB_tanh(s + 1)
                        # mm1(s+2) reuses the single ph1 bank; its first
                        # matmul must be emitted after relu(s+1) has read it.
                        if has_a and c >= 8:
                            for cc in range(3 * (c - 8), 3 * (c - 8) + 3):
                                phaseA_mm1(s + 2, cc)
                    prev_qt_map = qt_map
                s = SPC - 1
                sample_out_mu(s)
                store_last_half(s, 0)
                for c in range(NCH - 3, NCH):
                    if prev_qt_map[c] == "P":
                        phaseC_s2(s, c, eng="D")
                sample_out_sg1(s)
                sample_out_sg2(s, store=False)
                store_last_half(s, 1)
                del st[s]

            if loop_reps == 1:
                body()
            else:
                with tc.For_i(0, loop_reps, 1):
                    body()

    nc.compile()
    return nc


def _get_module(loop_reps=1):
    key = loop_reps
    if key not in _CACHE:
        _CACHE[key] = _build_module(loop_reps)
    return _CACHE[key]


def _host_prep(inputs):
    """Precompute folded parameters and shard inputs. Returns per-core in_maps."""
    x = np.asarray(inputs["x"])
    W1 = np.asarray(inputs["W1"], np.float32)
    b1 = np.asarray(inputs["b1"], np.float32)
    g1 = np.asarray(inputs["g1"], np.float32)
    beta1 = np.asarray(inputs["beta1"], np.float32)
    rm1 = np.asarray(inputs["rm1"], np.float32)
    rv1 = np.asarray(inputs["rv1"], np.float32)
    W2 = np.asarray(inputs["W2"], np.float32)
    b2 = np.asarray(inputs["b2"], np.float32)
    g2 = np.asarray(inputs["g2"], np.float32)
    rv2 = np.asarray(inputs["rv2"], np.float32)

    inv1 = (g1 / np.sqrt(rv1 + BN_EPS)).astype(np.float32)
    add1 = (beta1 - rm1 * inv1).astype(np.float32)
    inv2 = (g2 / np.sqrt(rv2 + BN_EPS)).astype(np.float32)
    # inv2 folded into W2; exp bias removed via per-channel threshold:
    # E' = exp(inv2*W2g); eb = max(E', exp(-inv2*b2)) — the exp(-inv2*b2)
    # factor cancels in mu/sg.
    thr = np.exp(-inv2 * b2).astype(np.float32)

    w1x_pm = W1[:, :C].T.reshape(NCH, 128, A).transpose(1, 0, 2)      # [128, NCH, A]
    wms_pm = W1[:, C:].T.reshape(2 * NCH, 128, A).transpose(1, 0, 2)  # [128, 2NCH, A]
    const = {
        "w1xT": np.ascontiguousarray(w1x_pm.reshape(128, NCH * A)).astype(ml_dtypes.bfloat16),
        "wmsT": np.ascontiguousarray(wms_pm.reshape(128, 2 * NCH * A)).astype(np.float32),
        "w2T": np.ascontiguousarray((W2 * inv2[:, None]).T).astype(ml_dtypes.bfloat16),
        "b1d": b1.reshape(A, 1),
        "inv1d": inv1.reshape(A, 1),
        "add1d": add1.reshape(A, 1),
        "thrd": np.ascontiguousarray(thr.reshape(NCH, 128).T),
        "identd": np.eye(128, dtype=np.float32),
    }
    xbf = x.astype(ml_dtypes.bfloat16)
    # partition-major: xr[s, p, c*T + t] = x[s, c*128 + p, t]
    xr = xbf.reshape(B, NCH, 128, T).transpose(0, 2, 1, 3).reshape(B, 128, NCH * T)
    in_maps = []
    for core in range(N_CORES):
        m = dict(const)
        m["xbf"] = np.ascontiguousarray(xr[core * SPC:(core + 1) * SPC])
        in_maps.append(m)
    return in_maps


def kernel(**inputs):
    from concourse.bass_utils import run_bass_kernel_spmd

    nc = _get_module(loop_reps=1)
    in_maps = _host_prep(inputs)
    res = run_bass_kernel_spmd(nc, in_maps, core_ids=list(range(N_CORES)))
    out = np.concatenate([res.results[i]["out"] for i in range(N_CORES)], axis=0)
    return out.astype(np.float32)
